# revision 1
# baseline (speedup 1.0000x reference)
"""Trainium2 Bass kernel: CentroidModule (VQ codebook update).

Strategy (data-parallel over B across 8 NeuronCores):
  - Each core gets 8192 tokens ([8 B-slices, 1024, 256] -> [8192, 256]).
  - Per 128-token tile:
      * normalize tokens: ss = sum(b^2) (ACT Square+accum), s_b = 1/sqrt(max(ss,1))
        (ACT relu/sqrt trick + DVE reciprocal), bn = b * s_b (DVE tensor_scalar).
      * PE-transpose bn -> bT (two 128x128 blocks via identity matmul).
      * scores = bn @ p_n^T on PE into PSUM [128 tok, 512 K] (2 accum steps).
      * One fused DVE tensor_tensor_reduce: t = scores + (-0.5*||p_n||^2)
        (broadcast row), accum = row max  -> argmax of t == argmin of d2.
      * one-hot A = (t >= m) via DVE tensor_scalar is_ge (fp32 0/1).
      * batchSums(+counts) = A^T @ [bn | 1] on PE, accumulated in PSUM over
        all 64 tiles (4 K-tiles x [128, 257]).
  - Per-core partial output [512, 257] (sums | counts); host reduces the 8
    partials and applies the tiny running-stat update + normalization.
"""

import os
import numpy as np
from contextlib import ExitStack

import concourse.bacc as bacc
import concourse.bass as bass
import concourse.mybir as mybir
import concourse.tile as tile
from concourse.bass_utils import run_bass_kernel_spmd

B, T, D, K = 64, 1024, 256, 512
NCORES = 8
TPC = (B * T) // NCORES      # tokens per core = 8192
NT = TPC // 128              # 64 token tiles per core
F32 = mybir.dt.float32
BF16 = mybir.dt.bfloat16
FP16 = mybir.dt.float16
AF = mybir.ActivationFunctionType
OP = mybir.AluOpType


def _body(tc, part_d, batch_d, protos_d, ident_d):
    nc = tc.nc
    with ExitStack() as ctx:
        const = ctx.enter_context(tc.tile_pool(name="const", bufs=1))
        work = ctx.enter_context(tc.tile_pool(name="work", bufs=4))
        small = ctx.enter_context(tc.tile_pool(name="small", bufs=6))
        ppt = ctx.enter_context(tc.tile_pool(name="ppt", bufs=3, space="PSUM"))
        ppb = ctx.enter_context(tc.tile_pool(name="ppb", bufs=1, space="PSUM"))
        psums = ctx.enter_context(tc.tile_pool(name="psums", bufs=1, space="PSUM"))

        ident = const.tile([128, 128], F32, tag="ident", name="ident")
        nc.sync.dma_start(ident[:], ident_d[:, :])
        neg1 = const.tile([128, 1], F32, tag="neg1", name="neg1")
        nc.gpsimd.memset(neg1[:], -1.0)

        # ---------------- proto prep (once per core) ----------------
        pnT0 = const.tile([128, K], F32, tag="pnT0", name="pnT0")
        pnT1 = const.tile([128, K], F32, tag="pnT1", name="pnT1")
        pnT = [pnT0, pnT1]
        halfneg = const.tile([128, 1], F32, tag="halfneg", name="halfneg")
        nc.gpsimd.memset(halfneg[:], -0.5)
        ones2 = const.tile([2, 128], BF16, tag="ones2", name="ones2")
        nc.gpsimd.memset(ones2[:], 1.0)

        for j in range(4):
            pk = const.tile([128, D], F32, tag="pk", bufs=2, name=f"pk{j}")
            nc.sync.dma_start(pk[:], protos_d[j * 128:(j + 1) * 128, :])
            pscr = const.tile([128, D], F32, tag="pscr", bufs=2, name=f"pscr{j}")
            ssp = small.tile([128, 1], F32, tag="ssp", name=f"ssp{j}")
            nc.scalar.activation(pscr[:], pk[:], AF.Square, accum_out=ssp[:])
            # s_p = 1 / sqrt(max(ss, 1)) = 1 / sqrt(relu(ss - 1) + 1)
            prel = small.tile([128, 1], F32, tag="prel", name=f"prel{j}")
            nc.scalar.activation(prel[:], ssp[:], AF.Relu, bias=neg1[:])
            plen = small.tile([128, 1], F32, tag="plen", name=f"plen{j}")
            nc.scalar.activation(plen[:], prel[:], AF.Sqrt, bias=1.0)
            sp = small.tile([128, 1], F32, tag="sp", name=f"sp{j}")
            nc.vector.reciprocal(sp[:], plen[:])
            pn = const.tile([128, D], F32, tag="pn", bufs=2, name=f"pn{j}")
            nc.vector.tensor_scalar_mul(pn[:], pk[:], sp[:])
            ptp = ppb.tile([128, D], F32, tag="btp", name=f"ptp{j}")
            for h in (0, 1):
                nc.tensor.transpose(
                    ptp[:, h * 128:(h + 1) * 128], pn[:, h * 128:(h + 1) * 128],
                    ident[:],
                )
                nc.vector.tensor_copy(
                    pnT[h][:, j * 128:(j + 1) * 128], ptp[:, h * 128:(h + 1) * 128]
                )

        # -0.5 * ||p_n||^2 as a [1, 512] row via matmul with a -0.5 column.
        pnsq0 = const.tile([128, K], F32, tag="pnsq0", name="pnsq0")
        pnsq1 = const.tile([128, K], F32, tag="pnsq1", name="pnsq1")
        nc.scalar.square(pnsq0[:], pnT0[:])
        nc.scalar.square(pnsq1[:], pnT1[:])
        pqps = ppt.tile([1, K], F32, tag="t", name="pqps")
        nc.tensor.matmul(pqps[:], lhsT=halfneg[:], rhs=pnsq0[:],
                         start=True, stop=False)
        nc.tensor.matmul(pqps[:], lhsT=halfneg[:], rhs=pnsq1[:],
                         start=False, stop=True)
        # bf16 hi/lo splits of p_n^T and the psq row (3-pass split-precision
        # matmul: hi*hi + lo*hi + hi*lo carries ~16 mantissa bits -> exact
        # argmax on this data, while every PE pass is a fast bf16 one).
        pnTh = [const.tile([128, K], BF16, tag=f"pnTh{h}", name=f"pnTh{h}")
                for h in (0, 1)]
        pnTl = [const.tile([128, K], BF16, tag=f"pnTl{h}", name=f"pnTl{h}")
                for h in (0, 1)]
        for h in (0, 1):
            nc.vector.tensor_copy(pnTh[h][:], pnT[h][:])
            nc.vector.tensor_sub(pnTl[h][:], pnT[h][:], pnTh[h][:])
        psqrh = const.tile([1, K], BF16, tag="psqrh", name="psqrh")
        psqrl = const.tile([1, K], BF16, tag="psqrl", name="psqrl")
        nc.vector.tensor_copy(psqrh[:], pqps[:])
        nc.vector.tensor_sub(psqrl[:], pqps[:], psqrh[:])
        # stack hi/lo rows into [2, K] so the psq bias is a single C=2 matmul
        psqr2 = const.tile([2, K], BF16, tag="psqr2", name="psqr2")
        nc.sync.dma_start(psqr2[0:1, :], psqrh[:])
        nc.sync.dma_start(psqr2[1:2, :], psqrl[:])

        # ---------------- accumulators ----------------
        acc = [
            psums.tile([128, D + 1], F32, tag=f"acc{kt}", name=f"acc{kt}")
            for kt in range(4)
        ]

        # ---------------- main loop: 4-stage skewed software pipeline ----
        # A(i): load + normalize; B(i): transpose; C(i): scores matmuls;
        # D(i): argmax one-hot + segment-sum matmuls.  Emitting A(i),
        # B(i-1), C(i-2), D(i-3) keeps every engine's program order free of
        # same-iteration chains, so iterations overlap ~3 deep.
        st = {}

        def stage_a(it):
            v = st.setdefault(it, {})
            bt = work.tile([128, D], F32, tag="bt", bufs=4, name=f"bt{it}")
            nc.sync.dma_start(bt[:], batch_d[it * 128:(it + 1) * 128, :])
            sq = work.tile([128, D], F32, tag="sq", bufs=3, name=f"sq{it}")
            ss = small.tile([128, 1], F32, tag="ss", name=f"ss{it}")
            nc.scalar.activation(sq[:], bt[:], AF.Square, accum_out=ss[:])
            ss1 = small.tile([128, 1], F32, tag="ss1", name=f"ss1{it}")
            nc.gpsimd.tensor_scalar_max(ss1[:], ss[:], 1.0)
            bln = small.tile([128, 1], F32, tag="bln", name=f"bln{it}")
            nc.scalar.activation(bln[:], ss1[:], AF.Sqrt)
            sb = small.tile([128, 1], F32, tag="sb", name=f"sb{it}")
            nc.vector.reciprocal(sb[:], bln[:])
            bn = work.tile([128, D + 1], F32, tag="bn", bufs=4, name=f"bn{it}")
            nc.vector.tensor_scalar_mul(bn[:, 0:D], bt[:], sb[:])
            nc.gpsimd.memset(bn[:, D:D + 1], 1.0)
            bnb = work.tile([128, D + 1], FP16, tag="bnb", bufs=8,
                            name=f"bnb{it}")
            nc.gpsimd.tensor_copy(bnb[:], bn[:])
            v["bn"], v["bnb"] = bn, bnb

        def stage_b(it):
            v = st[it]
            bn = v["bn"]
            btp = ppb.tile([128, D], F32, tag="btp", name=f"btp{it}")
            for h in (0, 1):
                nc.tensor.transpose(
                    btp[:, h * 128:(h + 1) * 128], bn[:, h * 128:(h + 1) * 128],
                    ident[:],
                )
            # bf16 hi/lo split of the transposed tokens, straight from PSUM
            bTh = work.tile([128, D], BF16, tag="bTh", bufs=6, name=f"bTh{it}")
            nc.vector.tensor_copy(bTh[:], btp[:])
            bTl = work.tile([128, D], BF16, tag="bTl", bufs=6, name=f"bTl{it}")
            nc.vector.tensor_sub(bTl[:], btp[:], bTh[:])
            v["bTh"], v["bTl"] = bTh, bTl

        def stage_c(it):
            v = st[it]
            bTh, bTl = v["bTh"], v["bTl"]
            # t = bn @ p_n^T - 0.5*||p_n||^2, all in split-precision bf16:
            # per d-tile hi*hi + lo*hi + hi*lo, plus a hi/lo C=1 psq bias.
            tps = ppt.tile([128, K], F32, tag="t", name=f"tps{it}")
            mms = []
            for h in (0, 1):
                s = slice(h * 128, (h + 1) * 128)
                mms += [(bTh[:, s], pnTh[h][:]), (bTl[:, s], pnTh[h][:]),
                        (bTh[:, s], pnTl[h][:])]
            mms += [(ones2[:], psqr2[:])]
            for i, (lhsT, rhs) in enumerate(mms):
                nc.tensor.matmul(tps[:], lhsT=lhsT, rhs=rhs,
                                 start=(i == 0), stop=(i == len(mms) - 1))
            v["tps"] = tps

        def stage_d(it):
            v = st.pop(it)
            tps, bnb = v["tps"], v["bnb"]
            mx = small.tile([128, 1], F32, tag="mx", name=f"mx{it}")
            nc.vector.reduce_max(mx[:], tps[:], axis=mybir.AxisListType.X)
            # A_raw = sign(m - t) in {0 (argmax), +1 (rest)}; fp16 is exact
            A = work.tile([128, K], FP16, tag="A", bufs=3, name=f"A{it}")
            nc.scalar.activation(A[:], tps[:], AF.Sign, bias=mx[:], scale=-1.0)
            for kt in range(4):
                nc.tensor.matmul(
                    acc[kt][:], lhsT=A[:, kt * 128:(kt + 1) * 128], rhs=bnb[:],
                    start=(it == 0), stop=(it == NT - 1),
                )

        for i in range(NT + 3):
            if i < NT:
                stage_a(i)
            if 0 <= i - 1 < NT:
                stage_b(i - 1)
            if 0 <= i - 2 < NT:
                stage_c(i - 2)
            if 0 <= i - 3 < NT:
                stage_d(i - 3)

        # ---------------- drain accumulators ----------------
        for kt in range(4):
            osb = work.tile([128, D + 1], F32, tag="osb", name=f"osb{kt}")
            nc.vector.tensor_copy(osb[:], acc[kt][:])
            nc.sync.dma_start(part_d[kt * 128:(kt + 1) * 128, :], osb[:])


def build_nc(debug=False):
    nc = bacc.Bacc("TRN2", target_bir_lowering=False, debug=debug,
                   num_devices=NCORES)
    batch_d = nc.dram_tensor("batch", [TPC, D], F32, kind="ExternalInput").ap()
    protos_d = nc.dram_tensor("protos", [K, D], F32, kind="ExternalInput").ap()
    ident_d = nc.dram_tensor("ident", [128, 128], F32, kind="ExternalInput").ap()
    part_d = nc.dram_tensor("partial", [K, D + 1], F32, kind="ExternalOutput").ap()
    with tile.TileContext(nc) as tc:
        _body(tc, part_d, batch_d, protos_d, ident_d)
    nc.compile()
    return nc


_NC_CACHE = {}


def _get_nc():
    if "nc" not in _NC_CACHE:
        _NC_CACHE["nc"] = build_nc()
    return _NC_CACHE["nc"]


def make_in_maps(batch, protos):
    flat = np.ascontiguousarray(batch.reshape(-1, D).astype(np.float32))
    ident = np.eye(128, dtype=np.float32)
    protos = np.ascontiguousarray(protos.astype(np.float32))
    return [
        {"batch": flat[i * TPC:(i + 1) * TPC], "protos": protos, "ident": ident}
        for i in range(NCORES)
    ]


def correct_partial(raw):
    """Device outputs raw[k] = sum_tok [tok not assigned to k] * bn[tok].
    True segment sums: sums[k] = total - raw[k], and sum_k raw = 511*total,
    so total = sum_k(raw)/511 exactly (in exact arithmetic)."""
    raw = np.asarray(raw, np.float64)
    tot = raw.sum(axis=0) / (K - 1)
    return tot[None, :] - raw


def finish(partials, protoSums, protoCounts):
    """Host-side all-reduce of per-core partials + running-stat update."""
    total = np.zeros((K, D + 1), np.float64)
    for p in partials:
        total += correct_partial(p)
    batchSums = total[:, :D]
    counts = total[:, D]
    newSums = protoSums.astype(np.float64) + batchSums
    newCounts = protoCounts.astype(np.float64) + counts
    newProtos = newSums / np.clip(newCounts, 1.0, None)[:, None]
    lens = np.sqrt(np.clip((newProtos * newProtos).sum(-1), 0.0, None))
    newProtos = newProtos / np.clip(lens, 1.0, None)[:, None]
    return newProtos.astype(np.float32)


def kernel(batch, protos, protoSums, protoCounts):
    nc = _get_nc()
    in_maps = make_in_maps(np.asarray(batch), np.asarray(protos))
    res = run_bass_kernel_spmd(nc, in_maps, list(range(NCORES)))
    partials = [r["partial"] for r in res.results]
    return finish(partials, np.asarray(protoSums), np.asarray(protoCounts))


if __name__ == "__main__":
    nc = build_nc()
    print("built + compiled OK")



# revision 12
# speedup vs baseline: 1.3034x; 1.3034x over previous
"""Trainium2 Bass kernel: CentroidModule (VQ codebook update).

Strategy (data-parallel over B across 8 NeuronCores):
  - Each core gets 8192 tokens fed as fp16 [8192, 256].
  - Per 128-token tile (9-deep skewed pipeline, no same-turn cross-engine
    chains):
      * Pool: sqd = bt*bt; DVE: ss = rowsum(sqd); Pool clamp; ACT sqrt;
        DVE reciprocal -> sb; DVE: bnb[:, :256] = bt * sb (fp16).
      * PE: 2x fp16 128x128 transposes -> ptp PSUM; ACT copy -> bT SBUF.
      * PE: t = bn @ pnT in ONE fp16 pass (2 matmuls, N=512) plus a C=2
        bf16 hi/lo bias matmul adding -0.5*||pn||^2 -> PSUM.
        (fp16 operand rounding gives ~3e-5 score noise vs ~2e-2 typical
        top-2 gap; tolerance is 2e-2 global so single-pass is safe.)
      * DVE: mx = row max of t (PSUM read).
      * ACT: A = Sign(-(t - mx)) in {0 (argmax), +1 (rest)}, fp16.
      * PE: batchSums(+counts) = A^T @ [bn | 1], 4 K-tiles accumulated in
        PSUM over all 64 tiles.
  - Per-core partial output [512, 257] (inverted sums | counts); host
    reduces the 8 partials and applies the tiny running-stat update.
"""

import numpy as np
from contextlib import ExitStack

import concourse.bacc as bacc
import concourse.bass as bass
import concourse.mybir as mybir
import concourse.tile as tile
from concourse.bass_utils import run_bass_kernel_spmd

B, T, D, K = 64, 1024, 256, 512
NCORES = 8
TPC = (B * T) // NCORES      # tokens per core = 8192
NT = TPC // 128              # 64 token tiles per core
F32 = mybir.dt.float32
BF16 = mybir.dt.bfloat16
FP16 = mybir.dt.float16
AF = mybir.ActivationFunctionType
OP = mybir.AluOpType


def _body(tc, part_d, batch_d, protos_d, ident_d):
    nc = tc.nc
    with ExitStack() as ctx:
        const = ctx.enter_context(tc.tile_pool(name="const", bufs=1))
        work = ctx.enter_context(tc.tile_pool(name="work", bufs=4))
        small = ctx.enter_context(tc.tile_pool(name="small", bufs=4))
        ppt = ctx.enter_context(tc.tile_pool(name="ppt", bufs=3, space="PSUM"))
        ppb = ctx.enter_context(tc.tile_pool(name="ppb", bufs=1, space="PSUM"))
        psums = ctx.enter_context(tc.tile_pool(name="psums", bufs=1, space="PSUM"))

        ident = const.tile([128, 128], FP16, tag="ident", name="ident")
        nc.sync.dma_start(ident[:], ident_d[:, :])
        halfneg = const.tile([128, 1], FP16, tag="halfneg", name="halfneg")
        nc.gpsimd.memset(halfneg[:], -0.5)
        ones2 = const.tile([2, 128], BF16, tag="ones2", name="ones2")
        nc.gpsimd.memset(ones2[:], 1.0)

        # ---------------- proto prep (once per core) ----------------
        pnT0 = const.tile([128, K], FP16, tag="pnT0", name="pnT0")
        pnT1 = const.tile([128, K], FP16, tag="pnT1", name="pnT1")
        pnT = [pnT0, pnT1]

        for j in range(4):
            pk = const.tile([128, D], F32, tag="pk", bufs=2, name=f"pk{j}")
            nc.sync.dma_start(pk[:], protos_d[j * 128:(j + 1) * 128, :])
            pscr = const.tile([128, D], FP16, tag="pscr", bufs=2, name=f"pscr{j}")
            ssp = small.tile([128, 1], F32, tag="ssp", name=f"ssp{j}")
            nc.scalar.activation(pscr[:], pk[:], AF.Square, accum_out=ssp[:])
            # s_p = 1 / sqrt(max(ss, 1))
            sspc = small.tile([128, 1], F32, tag="sspc", name=f"sspc{j}")
            nc.gpsimd.tensor_scalar_max(sspc[:], ssp[:], 1.0)
            spl = small.tile([128, 1], F32, tag="spl", name=f"spl{j}")
            nc.scalar.activation(spl[:], sspc[:], AF.Sqrt)
            sp = small.tile([128, 1], F32, tag="sp", name=f"sp{j}")
            nc.vector.reciprocal(sp[:], spl[:])
            pn = const.tile([128, D], FP16, tag="pn", bufs=2, name=f"pn{j}")
            nc.vector.tensor_scalar_mul(pn[:], pk[:], sp[:])
            ptp = ppb.tile([128, D], FP16, tag="ptp", name=f"ptp{j}")
            for h in (0, 1):
                nc.tensor.transpose(
                    ptp[:, h * 128:(h + 1) * 128], pn[:, h * 128:(h + 1) * 128],
                    ident[:],
                )
                nc.vector.tensor_copy(
                    pnT[h][:, j * 128:(j + 1) * 128], ptp[:, h * 128:(h + 1) * 128]
                )

        # bias row: -0.5 * ||pn_k||^2 as [1, 512] via matmul with a -0.5
        # column, then split into a bf16 hi/lo [2, 512] pair (C=2 matmul
        # later adds it into the scores PSUM exactly like the baseline).
        pnsq0 = const.tile([128, K], FP16, tag="pnsq0", name="pnsq0")
        pnsq1 = const.tile([128, K], FP16, tag="pnsq1", name="pnsq1")
        nc.scalar.square(pnsq0[:], pnT0[:])
        nc.scalar.square(pnsq1[:], pnT1[:])
        pqps = ppt.tile([1, K], F32, tag="tps", name="pqps")
        nc.tensor.matmul(pqps[:], lhsT=halfneg[:], rhs=pnsq0[:],
                         start=True, stop=False)
        nc.tensor.matmul(pqps[:], lhsT=halfneg[:], rhs=pnsq1[:],
                         start=False, stop=True)
        psqrh = const.tile([1, K], BF16, tag="psqrh", name="psqrh")
        psqrl = const.tile([1, K], BF16, tag="psqrl", name="psqrl")
        nc.vector.tensor_copy(psqrh[:], pqps[:])
        nc.vector.tensor_sub(psqrl[:], pqps[:], psqrh[:])
        psqr2 = const.tile([2, K], BF16, tag="psqr2", name="psqr2")
        nc.sync.dma_start(psqr2[0:1, :], psqrh[:])
        nc.sync.dma_start(psqr2[1:2, :], psqrl[:])

        # ---------------- accumulators ----------------
        acc = [
            psums.tile([128, D + 1], F32, tag=f"acc{kt}", name=f"acc{kt}")
            for kt in range(4)
        ]

        # ---------------- main loop: 9-stage skewed software pipeline ----
        st = {}

        def s_dma(it):
            v = st.setdefault(it, {})
            bt = work.tile([128, D], FP16, tag="bt", bufs=4, name=f"bt{it}")
            nc.sync.dma_start(bt[:], batch_d[it * 128:(it + 1) * 128, :])
            v["bt"] = bt

        def s_sq(it):
            v = st[it]
            sqd = work.tile([128, D], FP16, tag="sqd", bufs=3, name=f"sqd{it}")
            nc.gpsimd.tensor_mul(sqd[:], v["bt"][:], v["bt"][:])
            v["sqd"] = sqd

        def s_ssred(it):
            v = st[it]
            ss = small.tile([128, 1], F32, tag="ss", name=f"ss{it}")
            nc.vector.tensor_reduce(ss[:], v["sqd"][:], mybir.AxisListType.X,
                                    OP.add)
            ss1 = small.tile([128, 1], F32, tag="ss1", name=f"ss1{it}")
            nc.gpsimd.tensor_scalar_max(ss1[:], ss[:], 1.0)
            bln = small.tile([128, 1], F32, tag="bln", name=f"bln{it}")
            nc.scalar.activation(bln[:], ss1[:], AF.Sqrt)
            sb = small.tile([128, 1], F32, tag="sb", name=f"sb{it}")
            nc.vector.reciprocal(sb[:], bln[:])
            v["sb"] = sb

        def s_scale(it):
            v = st[it]
            bnb = work.tile([128, D + 1], FP16, tag="bnb", bufs=7,
                            name=f"bnb{it}")
            nc.vector.tensor_scalar_mul(bnb[:, 0:D], v["bt"][:], v["sb"][:])
            nc.gpsimd.memset(bnb[:, D:D + 1], 1.0)
            v["bnb"] = bnb

        def s_trans(it):
            v = st[it]
            bnb = v["bnb"]
            ptp = ppb.tile([128, D], FP16, tag="ptp", name=f"btp{it}")
            for h in (0, 1):
                nc.tensor.transpose(
                    ptp[:, h * 128:(h + 1) * 128], bnb[:, h * 128:(h + 1) * 128],
                    ident[:],
                )
            bT = work.tile([128, D], FP16, tag="bT", bufs=3, name=f"bT{it}")
            nc.scalar.activation(bT[:], ptp[:], AF.Copy)
            v["bT"] = bT

        def s_scores(it):
            v = st[it]
            bT = v["bT"]
            tps = ppt.tile([128, K], F32, tag="tps", name=f"tps{it}")
            for h in (0, 1):
                nc.tensor.matmul(tps[:], lhsT=bT[:, h * 128:(h + 1) * 128],
                                 rhs=pnT[h][:], start=(h == 0), stop=False)
            nc.tensor.matmul(tps[:], lhsT=ones2[:], rhs=psqr2[:],
                             start=False, stop=True)
            v["tps"] = tps

        def s_redmax(it):
            v = st[it]
            mx = small.tile([128, 1], F32, tag="mx", name=f"mx{it}")
            nc.vector.reduce_max(mx[:], v["tps"][:], axis=mybir.AxisListType.X)
            v["mx"] = mx

        def s_sign(it):
            v = st[it]
            A = work.tile([128, K], FP16, tag="A", bufs=3, name=f"A{it}")
            nc.scalar.activation(A[:], v["tps"][:], AF.Sign,
                                 bias=v["mx"][:], scale=-1.0)
            v["A"] = A

        def s_scatter(it):
            v = st.pop(it)
            A, bnb = v["A"], v["bnb"]
            for kt in range(4):
                nc.tensor.matmul(
                    acc[kt][:], lhsT=A[:, kt * 128:(kt + 1) * 128], rhs=bnb[:],
                    start=(it == 0), stop=(it == NT - 1),
                )

        stages = [s_dma, s_sq, s_ssred, s_scale, s_trans, s_scores,
                  s_redmax, s_sign, s_scatter]
        for j in range(NT + len(stages) - 1):
            for k, fn in enumerate(stages):
                if 0 <= j - k < NT:
                    fn(j - k)

        # ---------------- drain accumulators ----------------
        for kt in range(4):
            osb = work.tile([128, D + 1], F32, tag="osb", name=f"osb{kt}")
            nc.vector.tensor_copy(osb[:], acc[kt][:])
            nc.sync.dma_start(part_d[kt * 128:(kt + 1) * 128, :], osb[:])


def build_nc(debug=False):
    nc = bacc.Bacc("TRN2", target_bir_lowering=False, debug=debug,
                   num_devices=NCORES)
    batch_d = nc.dram_tensor("batch", [TPC, D], FP16, kind="ExternalInput").ap()
    protos_d = nc.dram_tensor("protos", [K, D], F32, kind="ExternalInput").ap()
    ident_d = nc.dram_tensor("ident", [128, 128], FP16, kind="ExternalInput").ap()
    part_d = nc.dram_tensor("partial", [K, D + 1], F32, kind="ExternalOutput").ap()
    with tile.TileContext(nc) as tc:
        _body(tc, part_d, batch_d, protos_d, ident_d)
    nc.compile()
    return nc


_NC_CACHE = {}


def _get_nc():
    if "nc" not in _NC_CACHE:
        _NC_CACHE["nc"] = build_nc()
    return _NC_CACHE["nc"]


def make_in_maps(batch, protos):
    flat = np.ascontiguousarray(
        batch.reshape(-1, D).astype(np.float16))
    ident = np.eye(128, dtype=np.float16)
    protos = np.ascontiguousarray(protos.astype(np.float32))
    return [
        {"batch": flat[i * TPC:(i + 1) * TPC], "protos": protos, "ident": ident}
        for i in range(NCORES)
    ]


def correct_partial(raw):
    """Device outputs raw[k] = sum_tok [tok not assigned to k] * bn[tok].
    True segment sums: sums[k] = total - raw[k], and sum_k raw = 511*total,
    so total = sum_k(raw)/511 exactly (in exact arithmetic)."""
    raw = np.asarray(raw, np.float64)
    tot = raw.sum(axis=0) / (K - 1)
    return tot[None, :] - raw


def finish(partials, protoSums, protoCounts):
    """Host-side all-reduce of per-core partials + running-stat update."""
    total = np.zeros((K, D + 1), np.float64)
    for p in partials:
        total += correct_partial(p)
    batchSums = total[:, :D]
    counts = total[:, D]
    newSums = protoSums.astype(np.float64) + batchSums
    newCounts = protoCounts.astype(np.float64) + counts
    newProtos = newSums / np.clip(newCounts, 1.0, None)[:, None]
    lens = np.sqrt(np.clip((newProtos * newProtos).sum(-1), 0.0, None))
    newProtos = newProtos / np.clip(lens, 1.0, None)[:, None]
    return newProtos.astype(np.float32)


def kernel(batch, protos, protoSums, protoCounts):
    nc = _get_nc()
    in_maps = make_in_maps(np.asarray(batch), np.asarray(protos))
    res = run_bass_kernel_spmd(nc, in_maps, list(range(NCORES)))
    partials = [r["partial"] for r in res.results]
    return finish(partials, np.asarray(protoSums), np.asarray(protoCounts))


if __name__ == "__main__":
    nc = build_nc()
    print("built + compiled OK")


# revision 14
# speedup vs baseline: 1.4004x; 1.0743x over previous
"""Trainium2 Bass kernel: CentroidModule (VQ codebook update).

Strategy (data-parallel over B across 8 NeuronCores):
  - Each core gets 8192 tokens fed as fp16 [8192, 256].
  - Per 128-token tile (9-deep skewed pipeline, no same-turn cross-engine
    chains):
      * Pool: sqd = bt*bt; DVE: ss = rowsum(sqd); Pool clamp; ACT sqrt;
        DVE reciprocal -> sb; DVE: bnb[:, :256] = bt * sb (fp16).
      * PE: 2x fp16 128x128 transposes -> ptp PSUM; ACT copy -> bT SBUF.
      * PE: t = bn @ pnT in ONE fp16 pass (2 matmuls, N=512) plus a C=2
        bf16 hi/lo bias matmul adding -0.5*||pn||^2 -> PSUM.
        (fp16 operand rounding gives ~3e-5 score noise vs ~2e-2 typical
        top-2 gap; tolerance is 2e-2 global so single-pass is safe.)
      * DVE: mx = row max of t (PSUM read).
      * ACT: A = Sign(-(t - mx)) in {0 (argmax), +1 (rest)}, fp16.
      * PE: batchSums(+counts) = A^T @ [bn | 1], 4 K-tiles accumulated in
        PSUM over all 64 tiles.
  - Per-core partial output [512, 257] (inverted sums | counts); host
    reduces the 8 partials and applies the tiny running-stat update.
"""

import numpy as np
from contextlib import ExitStack

import concourse.bacc as bacc
import concourse.bass as bass
import concourse.mybir as mybir
import concourse.tile as tile
from concourse.bass_utils import run_bass_kernel_spmd

B, T, D, K = 64, 1024, 256, 512
NCORES = 8
TPC = (B * T) // NCORES      # tokens per core = 8192
NT = TPC // 128              # 64 token tiles per core
F32 = mybir.dt.float32
BF16 = mybir.dt.bfloat16
FP16 = mybir.dt.float16
AF = mybir.ActivationFunctionType
OP = mybir.AluOpType


def _body(tc, part_d, batch_d, protos_d, ident_d):
    nc = tc.nc
    with ExitStack() as ctx:
        const = ctx.enter_context(tc.tile_pool(name="const", bufs=1))
        work = ctx.enter_context(tc.tile_pool(name="work", bufs=4))
        small = ctx.enter_context(tc.tile_pool(name="small", bufs=4))
        ppt = ctx.enter_context(tc.tile_pool(name="ppt", bufs=3, space="PSUM"))
        ppb = ctx.enter_context(tc.tile_pool(name="ppb", bufs=1, space="PSUM"))
        psums = ctx.enter_context(tc.tile_pool(name="psums", bufs=1, space="PSUM"))

        ident = const.tile([128, 128], FP16, tag="ident", name="ident")
        nc.sync.dma_start(ident[:], ident_d[:, :])
        halfneg = const.tile([128, 1], FP16, tag="halfneg", name="halfneg")
        nc.gpsimd.memset(halfneg[:], -0.5)
        ones2 = const.tile([2, 128], BF16, tag="ones2", name="ones2")
        nc.gpsimd.memset(ones2[:], 1.0)

        # HAM pre-warm: ~3.4us of dummy back-to-back matmuls during the
        # DMA-bound prologue so the PE clock-gate opens (1.2 -> 2.4 GHz)
        # before the main loop starts.
        wsrc = const.tile([128, 128], FP16, tag="wsrc", name="wsrc")
        nc.gpsimd.memset(wsrc[:], 0.0)
        for w in range(32):
            wps = ppt.tile([128, 128], F32, tag="tps", name=f"warm{w}")
            nc.tensor.matmul(wps[:], lhsT=wsrc[:], rhs=wsrc[:],
                             start=True, stop=True)

        # ---------------- proto prep (once per core) ----------------
        pnT0 = const.tile([128, K], FP16, tag="pnT0", name="pnT0")
        pnT1 = const.tile([128, K], FP16, tag="pnT1", name="pnT1")
        pnT = [pnT0, pnT1]

        for j in range(4):
            pk = const.tile([128, D], F32, tag="pk", bufs=2, name=f"pk{j}")
            nc.sync.dma_start(pk[:], protos_d[j * 128:(j + 1) * 128, :])
            pscr = const.tile([128, D], FP16, tag="pscr", bufs=2, name=f"pscr{j}")
            ssp = small.tile([128, 1], F32, tag="ssp", name=f"ssp{j}")
            nc.scalar.activation(pscr[:], pk[:], AF.Square, accum_out=ssp[:])
            # s_p = 1 / sqrt(max(ss, 1))
            sspc = small.tile([128, 1], F32, tag="sspc", name=f"sspc{j}")
            nc.gpsimd.tensor_scalar_max(sspc[:], ssp[:], 1.0)
            spl = small.tile([128, 1], F32, tag="spl", name=f"spl{j}")
            nc.scalar.activation(spl[:], sspc[:], AF.Sqrt)
            sp = small.tile([128, 1], F32, tag="sp", name=f"sp{j}")
            nc.vector.reciprocal(sp[:], spl[:])
            pn = const.tile([128, D], FP16, tag="pn", bufs=2, name=f"pn{j}")
            nc.vector.tensor_scalar_mul(pn[:], pk[:], sp[:])
            ptp = ppb.tile([128, D], FP16, tag="ptp", name=f"ptp{j}")
            for h in (0, 1):
                nc.tensor.transpose(
                    ptp[:, h * 128:(h + 1) * 128], pn[:, h * 128:(h + 1) * 128],
                    ident[:],
                )
                nc.vector.tensor_copy(
                    pnT[h][:, j * 128:(j + 1) * 128], ptp[:, h * 128:(h + 1) * 128]
                )

        # bias row: -0.5 * ||pn_k||^2 as [1, 512] via matmul with a -0.5
        # column, then split into a bf16 hi/lo [2, 512] pair (C=2 matmul
        # later adds it into the scores PSUM exactly like the baseline).
        pnsq0 = const.tile([128, K], FP16, tag="pnsq0", name="pnsq0")
        pnsq1 = const.tile([128, K], FP16, tag="pnsq1", name="pnsq1")
        nc.scalar.square(pnsq0[:], pnT0[:])
        nc.scalar.square(pnsq1[:], pnT1[:])
        pqps = ppt.tile([1, K], F32, tag="tps", name="pqps")
        nc.tensor.matmul(pqps[:], lhsT=halfneg[:], rhs=pnsq0[:],
                         start=True, stop=False)
        nc.tensor.matmul(pqps[:], lhsT=halfneg[:], rhs=pnsq1[:],
                         start=False, stop=True)
        psqrh = const.tile([1, K], BF16, tag="psqrh", name="psqrh")
        psqrl = const.tile([1, K], BF16, tag="psqrl", name="psqrl")
        nc.vector.tensor_copy(psqrh[:], pqps[:])
        nc.vector.tensor_sub(psqrl[:], pqps[:], psqrh[:])
        psqr2 = const.tile([2, K], BF16, tag="psqr2", name="psqr2")
        nc.sync.dma_start(psqr2[0:1, :], psqrh[:])
        nc.sync.dma_start(psqr2[1:2, :], psqrl[:])

        # ---------------- accumulators ----------------
        acc = [
            psums.tile([128, D + 1], F32, tag=f"acc{kt}", name=f"acc{kt}")
            for kt in range(4)
        ]

        # ---------------- main loop: 9-deep skewed software pipeline ----
        # Stage offsets (tile i's op runs at turn i + offset):
        #   dma 0 | sq 1 | ssred/sqrt/recip 2 | scale 3 | trans+btcopy 4 |
        #   scores 5 | redmax 6 | sign 7 | scatter 8
        # Within a turn, each engine's ops are emitted with
        # satisfied-dependency ops FIRST and ops that wait on same-turn
        # producers LAST, so no in-order engine queue head-blocks.
        st = {}

        def live(j, k):
            return 0 <= j - k < NT

        for j in range(NT + 9):
            # ---- DMA ----
            if live(j, 0):
                it = j
                v = st.setdefault(it, {})
                bt = work.tile([128, D], FP16, tag="bt", bufs=4, name=f"bt{it}")
                nc.sync.dma_start(bt[:], batch_d[it * 128:(it + 1) * 128, :])
                v["bt"] = bt

            # ---- PE queue (all deps >= 1 turn old except btcopy below) ----
            if live(j, 4):
                it = j - 4
                v = st[it]
                ptp = ppb.tile([128, D], FP16, tag="ptp", name=f"btp{it}")
                for h in (0, 1):
                    nc.tensor.transpose(
                        ptp[:, h * 128:(h + 1) * 128],
                        v["bnb"][:, h * 128:(h + 1) * 128], ident[:],
                    )
                v["ptp"] = ptp
            if live(j, 5):
                it = j - 5
                v = st[it]
                tps = ppt.tile([128, K], F32, tag="tps", name=f"tps{it}")
                for h in (0, 1):
                    nc.tensor.matmul(tps[:], lhsT=v["bT"][:, h * 128:(h + 1) * 128],
                                     rhs=pnT[h][:], start=(h == 0), stop=False)
                nc.tensor.matmul(tps[:], lhsT=ones2[:], rhs=psqr2[:],
                                 start=False, stop=True)
                v["tps"] = tps
            if live(j, 8):
                it = j - 8
                v = st.pop(it)
                for kt in range(4):
                    nc.tensor.matmul(
                        acc[kt][:], lhsT=v["A"][:, kt * 128:(kt + 1) * 128],
                        rhs=v["bnb"][:],
                        start=(it == 0), stop=(it == NT - 1),
                    )

            # ---- DVE queue: redmax, scale, ssred (old deps), recip last ----
            if live(j, 6):
                it = j - 6
                v = st[it]
                mx = small.tile([128, 1], F32, tag="mx", name=f"mx{it}")
                nc.vector.reduce_max(mx[:], v["tps"][:],
                                     axis=mybir.AxisListType.X)
                v["mx"] = mx
            if live(j, 3):
                it = j - 3
                v = st[it]
                bnb = work.tile([128, D + 1], FP16, tag="bnb", bufs=7,
                                name=f"bnb{it}")
                nc.vector.tensor_scalar_mul(bnb[:, 0:D], v["bt"][:], v["sb"][:])
                v["bnb"] = bnb
            if live(j, 2):
                it = j - 2
                v = st[it]
                ss = small.tile([128, 1], F32, tag="ss", name=f"ss{it}")
                nc.vector.tensor_reduce(ss[:], v["sqd"][:],
                                        mybir.AxisListType.X, OP.add)
                v["ss"] = ss

            # ---- Pool queue ----
            if live(j, 1):
                it = j - 1
                v = st[it]
                sqd = work.tile([128, D], FP16, tag="sqd", bufs=3,
                                name=f"sqd{it}")
                nc.gpsimd.tensor_mul(sqd[:], v["bt"][:], v["bt"][:])
                v["sqd"] = sqd
            if live(j, 3):
                nc.gpsimd.memset(st[j - 3]["bnb"][:, D:D + 1], 1.0)
            if live(j, 2):
                it = j - 2
                v = st[it]
                ss1 = small.tile([128, 1], F32, tag="ss1", name=f"ss1{it}")
                nc.gpsimd.tensor_scalar_max(ss1[:], v["ss"][:], 1.0)
                v["ss1"] = ss1

            # ---- ACT queue: sign (old deps) first, btcopy, sqrt last ----
            if live(j, 7):
                it = j - 7
                v = st[it]
                A = work.tile([128, K], FP16, tag="A", bufs=3, name=f"A{it}")
                nc.scalar.activation(A[:], v["tps"][:], AF.Sign,
                                     bias=v["mx"][:], scale=-1.0)
                v["A"] = A
            if live(j, 4):
                it = j - 4
                v = st[it]
                bT = work.tile([128, D], FP16, tag="bT", bufs=3, name=f"bT{it}")
                nc.scalar.activation(bT[:], v["ptp"][:], AF.Copy)
                v["bT"] = bT
            if live(j, 2):
                it = j - 2
                v = st[it]
                bln = small.tile([128, 1], F32, tag="bln", name=f"bln{it}")
                nc.scalar.activation(bln[:], v["ss1"][:], AF.Sqrt)
                v["bln"] = bln

            # ---- DVE tail: recip (waits on same-turn ACT sqrt) ----
            if live(j, 2):
                it = j - 2
                v = st[it]
                sb = small.tile([128, 1], F32, tag="sb", name=f"sb{it}")
                nc.vector.reciprocal(sb[:], v["bln"][:])
                v["sb"] = sb

        # ---------------- drain accumulators ----------------
        for kt in range(4):
            osb = work.tile([128, D + 1], F32, tag="osb", name=f"osb{kt}")
            nc.vector.tensor_copy(osb[:], acc[kt][:])
            nc.sync.dma_start(part_d[kt * 128:(kt + 1) * 128, :], osb[:])


def build_nc(debug=False):
    nc = bacc.Bacc("TRN2", target_bir_lowering=False, debug=debug,
                   num_devices=NCORES)
    batch_d = nc.dram_tensor("batch", [TPC, D], FP16, kind="ExternalInput").ap()
    protos_d = nc.dram_tensor("protos", [K, D], F32, kind="ExternalInput").ap()
    ident_d = nc.dram_tensor("ident", [128, 128], FP16, kind="ExternalInput").ap()
    part_d = nc.dram_tensor("partial", [K, D + 1], F32, kind="ExternalOutput").ap()
    with tile.TileContext(nc) as tc:
        _body(tc, part_d, batch_d, protos_d, ident_d)
    nc.compile()
    return nc


_NC_CACHE = {}


def _get_nc():
    if "nc" not in _NC_CACHE:
        _NC_CACHE["nc"] = build_nc()
    return _NC_CACHE["nc"]


def make_in_maps(batch, protos):
    flat = np.ascontiguousarray(
        batch.reshape(-1, D).astype(np.float16))
    ident = np.eye(128, dtype=np.float16)
    protos = np.ascontiguousarray(protos.astype(np.float32))
    return [
        {"batch": flat[i * TPC:(i + 1) * TPC], "protos": protos, "ident": ident}
        for i in range(NCORES)
    ]


def correct_partial(raw):
    """Device outputs raw[k] = sum_tok [tok not assigned to k] * bn[tok].
    True segment sums: sums[k] = total - raw[k], and sum_k raw = 511*total,
    so total = sum_k(raw)/511 exactly (in exact arithmetic)."""
    raw = np.asarray(raw, np.float64)
    tot = raw.sum(axis=0) / (K - 1)
    return tot[None, :] - raw


def finish(partials, protoSums, protoCounts):
    """Host-side all-reduce of per-core partials + running-stat update."""
    total = np.zeros((K, D + 1), np.float64)
    for p in partials:
        total += correct_partial(p)
    batchSums = total[:, :D]
    counts = total[:, D]
    newSums = protoSums.astype(np.float64) + batchSums
    newCounts = protoCounts.astype(np.float64) + counts
    newProtos = newSums / np.clip(newCounts, 1.0, None)[:, None]
    lens = np.sqrt(np.clip((newProtos * newProtos).sum(-1), 0.0, None))
    newProtos = newProtos / np.clip(lens, 1.0, None)[:, None]
    return newProtos.astype(np.float32)


def kernel(batch, protos, protoSums, protoCounts):
    nc = _get_nc()
    in_maps = make_in_maps(np.asarray(batch), np.asarray(protos))
    res = run_bass_kernel_spmd(nc, in_maps, list(range(NCORES)))
    partials = [r["partial"] for r in res.results]
    return finish(partials, np.asarray(protoSums), np.asarray(protoCounts))


if __name__ == "__main__":
    nc = build_nc()
    print("built + compiled OK")


# revision 16
# speedup vs baseline: 1.4085x; 1.0058x over previous
"""Trainium2 Bass kernel: CentroidModule (VQ codebook update).

Strategy (data-parallel over B across 8 NeuronCores):
  - Each core gets 8192 tokens fed as fp16 [8192, 256].
  - Per 128-token tile (9-deep skewed pipeline, no same-turn cross-engine
    chains):
      * Pool: sqd = bt*bt; DVE: ss = rowsum(sqd); Pool clamp; ACT sqrt;
        DVE reciprocal -> sb; DVE: bnb[:, :256] = bt * sb (fp16).
      * PE: 2x fp16 128x128 transposes -> ptp PSUM; ACT copy -> bT SBUF.
      * PE: t = bn @ pnT in ONE fp16 pass (2 matmuls, N=512) plus a C=2
        bf16 hi/lo bias matmul adding -0.5*||pn||^2 -> PSUM.
        (fp16 operand rounding gives ~3e-5 score noise vs ~2e-2 typical
        top-2 gap; tolerance is 2e-2 global so single-pass is safe.)
      * DVE: mx = row max of t (PSUM read).
      * ACT: A = Sign(-(t - mx)) in {0 (argmax), +1 (rest)}, fp16.
      * PE: batchSums(+counts) = A^T @ [bn | 1], 4 K-tiles accumulated in
        PSUM over all 64 tiles.
  - Per-core partial output [512, 257] (inverted sums | counts); host
    reduces the 8 partials and applies the tiny running-stat update.
"""

import numpy as np
from contextlib import ExitStack

import concourse.bacc as bacc
import concourse.bass as bass
import concourse.mybir as mybir
import concourse.tile as tile
from concourse.bass_utils import run_bass_kernel_spmd

B, T, D, K = 64, 1024, 256, 512
NCORES = 8
TPC = (B * T) // NCORES      # tokens per core = 8192
NT = TPC // 128              # 64 token tiles per core
F32 = mybir.dt.float32
BF16 = mybir.dt.bfloat16
FP16 = mybir.dt.float16
AF = mybir.ActivationFunctionType
OP = mybir.AluOpType


def _body(tc, part_d, batch_d, protos_d, ident_d):
    nc = tc.nc
    with ExitStack() as ctx:
        const = ctx.enter_context(tc.tile_pool(name="const", bufs=1))
        work = ctx.enter_context(tc.tile_pool(name="work", bufs=4))
        small = ctx.enter_context(tc.tile_pool(name="small", bufs=4))
        ppt = ctx.enter_context(tc.tile_pool(name="ppt", bufs=3, space="PSUM"))
        ppb = ctx.enter_context(tc.tile_pool(name="ppb", bufs=1, space="PSUM"))
        psums = ctx.enter_context(tc.tile_pool(name="psums", bufs=1, space="PSUM"))

        ident = const.tile([128, 128], FP16, tag="ident", name="ident")
        nc.sync.dma_start(ident[:], ident_d[:, :])
        halfneg = const.tile([128, 1], FP16, tag="halfneg", name="halfneg")
        nc.gpsimd.memset(halfneg[:], -0.5)
        ones2 = const.tile([2, 128], BF16, tag="ones2", name="ones2")
        nc.gpsimd.memset(ones2[:], 1.0)

        # HAM pre-warm: ~3.4us of dummy back-to-back matmuls during the
        # DMA-bound prologue so the PE clock-gate opens (1.2 -> 2.4 GHz)
        # before the main loop starts.
        wsrc = const.tile([128, 128], FP16, tag="wsrc", name="wsrc")
        nc.gpsimd.memset(wsrc[:], 0.0)
        wsrc2 = const.tile([128, K], FP16, tag="wsrc2", name="wsrc2")
        nc.gpsimd.memset(wsrc2[:], 0.0)
        for w in range(9):
            wps = ppt.tile([128, K], F32, tag="tps", name=f"warm{w}")
            nc.tensor.matmul(wps[:], lhsT=wsrc[:], rhs=wsrc2[:],
                             start=True, stop=True)

        # ---------------- proto prep (once per core) ----------------
        pnT0 = const.tile([128, K], FP16, tag="pnT0", name="pnT0")
        pnT1 = const.tile([128, K], FP16, tag="pnT1", name="pnT1")
        pnT = [pnT0, pnT1]

        for j in range(4):
            pk = const.tile([128, D], F32, tag="pk", bufs=2, name=f"pk{j}")
            nc.sync.dma_start(pk[:], protos_d[j * 128:(j + 1) * 128, :])
            pscr = const.tile([128, D], FP16, tag="pscr", bufs=2, name=f"pscr{j}")
            ssp = small.tile([128, 1], F32, tag="ssp", name=f"ssp{j}")
            nc.scalar.activation(pscr[:], pk[:], AF.Square, accum_out=ssp[:])
            # s_p = 1 / sqrt(max(ss, 1))
            sspc = small.tile([128, 1], F32, tag="sspc", name=f"sspc{j}")
            nc.gpsimd.tensor_scalar_max(sspc[:], ssp[:], 1.0)
            spl = small.tile([128, 1], F32, tag="spl", name=f"spl{j}")
            nc.scalar.activation(spl[:], sspc[:], AF.Sqrt)
            sp = small.tile([128, 1], F32, tag="sp", name=f"sp{j}")
            nc.vector.reciprocal(sp[:], spl[:])
            pn = const.tile([128, D], FP16, tag="pn", bufs=2, name=f"pn{j}")
            nc.vector.tensor_scalar_mul(pn[:], pk[:], sp[:])
            ptp = ppb.tile([128, D], FP16, tag="ptp", name=f"ptp{j}")
            for h in (0, 1):
                nc.tensor.transpose(
                    ptp[:, h * 128:(h + 1) * 128], pn[:, h * 128:(h + 1) * 128],
                    ident[:],
                )
                nc.vector.tensor_copy(
                    pnT[h][:, j * 128:(j + 1) * 128], ptp[:, h * 128:(h + 1) * 128]
                )

        # bias row: -0.5 * ||pn_k||^2 as [1, 512] via matmul with a -0.5
        # column, then split into a bf16 hi/lo [2, 512] pair (C=2 matmul
        # later adds it into the scores PSUM exactly like the baseline).
        pnsq0 = const.tile([128, K], FP16, tag="pnsq0", name="pnsq0")
        pnsq1 = const.tile([128, K], FP16, tag="pnsq1", name="pnsq1")
        nc.scalar.square(pnsq0[:], pnT0[:])
        nc.scalar.square(pnsq1[:], pnT1[:])
        pqps = ppt.tile([1, K], F32, tag="tps", name="pqps")
        nc.tensor.matmul(pqps[:], lhsT=halfneg[:], rhs=pnsq0[:],
                         start=True, stop=False)
        nc.tensor.matmul(pqps[:], lhsT=halfneg[:], rhs=pnsq1[:],
                         start=False, stop=True)
        psqrh = const.tile([1, K], BF16, tag="psqrh", name="psqrh")
        psqrl = const.tile([1, K], BF16, tag="psqrl", name="psqrl")
        nc.vector.tensor_copy(psqrh[:], pqps[:])
        nc.vector.tensor_sub(psqrl[:], pqps[:], psqrh[:])
        psqr2 = const.tile([2, K], BF16, tag="psqr2", name="psqr2")
        nc.sync.dma_start(psqr2[0:1, :], psqrh[:])
        nc.sync.dma_start(psqr2[1:2, :], psqrl[:])

        # ---------------- accumulators ----------------
        acc = [
            psums.tile([128, D + 1], F32, tag=f"acc{kt}", name=f"acc{kt}")
            for kt in range(4)
        ]

        # ---------------- main loop: 9-deep skewed software pipeline ----
        # Stage offsets (tile i's op runs at turn i + offset):
        #   dma 0 | sq 1 | ssred/sqrt/recip 2 | scale 3 | trans+btcopy 4 |
        #   scores 5 | redmax 6 | sign 7 | scatter 8
        # Within a turn, each engine's ops are emitted with
        # satisfied-dependency ops FIRST and ops that wait on same-turn
        # producers LAST, so no in-order engine queue head-blocks.
        st = {}

        def live(j, k):
            return 0 <= j - k < NT

        for j in range(NT + 9):
            # ---- DMA ----
            if live(j, 0):
                it = j
                v = st.setdefault(it, {})
                bt = work.tile([128, D], FP16, tag="bt", bufs=4, name=f"bt{it}")
                nc.sync.dma_start(bt[:], batch_d[it * 128:(it + 1) * 128, :])
                v["bt"] = bt

            # ---- PE queue: short scatter MMs first (their LDWs pull
            # ahead under the previous turn's long scores MMs), then
            # transposes, then the 3 long scores MMs. ----
            if live(j, 8):
                it = j - 8
                v = st.pop(it)
                for kt in range(4):
                    nc.tensor.matmul(
                        acc[kt][:], lhsT=v["A"][:, kt * 128:(kt + 1) * 128],
                        rhs=v["bnb"][:],
                        start=(it == 0), stop=(it == NT - 1),
                    )
            if live(j, 4):
                it = j - 4
                v = st[it]
                ptp = ppb.tile([128, D], FP16, tag="ptp", name=f"btp{it}")
                for h in (0, 1):
                    nc.tensor.transpose(
                        ptp[:, h * 128:(h + 1) * 128],
                        v["bnb"][:, h * 128:(h + 1) * 128], ident[:],
                    )
                v["ptp"] = ptp
            if live(j, 5):
                it = j - 5
                v = st[it]
                tps = ppt.tile([128, K], F32, tag="tps", name=f"tps{it}")
                nc.tensor.matmul(tps[:], lhsT=ones2[:], rhs=psqr2[:],
                                 start=True, stop=False)
                for h in (0, 1):
                    nc.tensor.matmul(tps[:], lhsT=v["bT"][:, h * 128:(h + 1) * 128],
                                     rhs=pnT[h][:], start=False, stop=(h == 1))
                v["tps"] = tps

            # ---- DVE queue: redmax, scale, ssred (old deps), recip last ----
            if live(j, 6):
                it = j - 6
                v = st[it]
                mx = small.tile([128, 1], F32, tag="mx", name=f"mx{it}")
                nc.vector.reduce_max(mx[:], v["tps"][:],
                                     axis=mybir.AxisListType.X)
                v["mx"] = mx
            if live(j, 3):
                it = j - 3
                v = st[it]
                bnb = work.tile([128, D + 1], FP16, tag="bnb", bufs=7,
                                name=f"bnb{it}")
                nc.vector.tensor_scalar_mul(bnb[:, 0:D], v["bt"][:], v["sb"][:])
                v["bnb"] = bnb
            if live(j, 2):
                it = j - 2
                v = st[it]
                ss = small.tile([128, 1], F32, tag="ss", name=f"ss{it}")
                nc.vector.tensor_reduce(ss[:], v["sqd"][:],
                                        mybir.AxisListType.X, OP.add)
                v["ss"] = ss

            # ---- Pool queue ----
            if live(j, 1):
                it = j - 1
                v = st[it]
                sqd = work.tile([128, D], FP16, tag="sqd", bufs=3,
                                name=f"sqd{it}")
                nc.gpsimd.tensor_mul(sqd[:], v["bt"][:], v["bt"][:])
                v["sqd"] = sqd
            if live(j, 3):
                nc.gpsimd.memset(st[j - 3]["bnb"][:, D:D + 1], 1.0)
            if live(j, 2):
                it = j - 2
                v = st[it]
                ss1 = small.tile([128, 1], F32, tag="ss1", name=f"ss1{it}")
                nc.gpsimd.tensor_scalar_max(ss1[:], v["ss"][:], 1.0)
                v["ss1"] = ss1

            # ---- ACT queue: sign (old deps) first, btcopy, sqrt last ----
            if live(j, 7):
                it = j - 7
                v = st[it]
                A = work.tile([128, K], FP16, tag="A", bufs=3, name=f"A{it}")
                nc.scalar.activation(A[:], v["tps"][:], AF.Sign,
                                     bias=v["mx"][:], scale=-1.0)
                v["A"] = A
            if live(j, 4):
                it = j - 4
                v = st[it]
                bT = work.tile([128, D], FP16, tag="bT", bufs=3, name=f"bT{it}")
                nc.scalar.activation(bT[:], v["ptp"][:], AF.Copy)
                v["bT"] = bT
            if live(j, 2):
                it = j - 2
                v = st[it]
                bln = small.tile([128, 1], F32, tag="bln", name=f"bln{it}")
                nc.scalar.activation(bln[:], v["ss1"][:], AF.Sqrt)
                v["bln"] = bln

            # ---- DVE tail: recip (waits on same-turn ACT sqrt) ----
            if live(j, 2):
                it = j - 2
                v = st[it]
                sb = small.tile([128, 1], F32, tag="sb", name=f"sb{it}")
                nc.vector.reciprocal(sb[:], v["bln"][:])
                v["sb"] = sb

        # ---------------- drain accumulators ----------------
        for kt in range(4):
            osb = work.tile([128, D + 1], F32, tag="osb", name=f"osb{kt}")
            nc.vector.tensor_copy(osb[:], acc[kt][:])
            nc.sync.dma_start(part_d[kt * 128:(kt + 1) * 128, :], osb[:])


def build_nc(debug=False):
    nc = bacc.Bacc("TRN2", target_bir_lowering=False, debug=debug,
                   num_devices=NCORES)
    batch_d = nc.dram_tensor("batch", [TPC, D], FP16, kind="ExternalInput").ap()
    protos_d = nc.dram_tensor("protos", [K, D], F32, kind="ExternalInput").ap()
    ident_d = nc.dram_tensor("ident", [128, 128], FP16, kind="ExternalInput").ap()
    part_d = nc.dram_tensor("partial", [K, D + 1], F32, kind="ExternalOutput").ap()
    with tile.TileContext(nc) as tc:
        _body(tc, part_d, batch_d, protos_d, ident_d)
    nc.compile()
    return nc


_NC_CACHE = {}


def _get_nc():
    if "nc" not in _NC_CACHE:
        _NC_CACHE["nc"] = build_nc()
    return _NC_CACHE["nc"]


def make_in_maps(batch, protos):
    flat = np.ascontiguousarray(
        batch.reshape(-1, D).astype(np.float16))
    ident = np.eye(128, dtype=np.float16)
    protos = np.ascontiguousarray(protos.astype(np.float32))
    return [
        {"batch": flat[i * TPC:(i + 1) * TPC], "protos": protos, "ident": ident}
        for i in range(NCORES)
    ]


def correct_partial(raw):
    """Device outputs raw[k] = sum_tok [tok not assigned to k] * bn[tok].
    True segment sums: sums[k] = total - raw[k], and sum_k raw = 511*total,
    so total = sum_k(raw)/511 exactly (in exact arithmetic)."""
    raw = np.asarray(raw, np.float64)
    tot = raw.sum(axis=0) / (K - 1)
    return tot[None, :] - raw


def finish(partials, protoSums, protoCounts):
    """Host-side all-reduce of per-core partials + running-stat update."""
    total = np.zeros((K, D + 1), np.float64)
    for p in partials:
        total += correct_partial(p)
    batchSums = total[:, :D]
    counts = total[:, D]
    newSums = protoSums.astype(np.float64) + batchSums
    newCounts = protoCounts.astype(np.float64) + counts
    newProtos = newSums / np.clip(newCounts, 1.0, None)[:, None]
    lens = np.sqrt(np.clip((newProtos * newProtos).sum(-1), 0.0, None))
    newProtos = newProtos / np.clip(lens, 1.0, None)[:, None]
    return newProtos.astype(np.float32)


def kernel(batch, protos, protoSums, protoCounts):
    nc = _get_nc()
    in_maps = make_in_maps(np.asarray(batch), np.asarray(protos))
    res = run_bass_kernel_spmd(nc, in_maps, list(range(NCORES)))
    partials = [r["partial"] for r in res.results]
    return finish(partials, np.asarray(protoSums), np.asarray(protoCounts))


if __name__ == "__main__":
    nc = build_nc()
    print("built + compiled OK")
